# revision 59
# baseline (speedup 1.0000x reference)
"""Trainium2 Bass kernel for a batched Kalman filter.

Math: the covariance/gain recursion of the Kalman filter is independent of the
measurements and the initial covariance is identical for every batch element,
so the per-step gain K_t and transition A_t = (I - K_t H) F are batch-uniform
and computed once on the host (float64). The filtered states are then a pure
linear function of (x0, z):

    x_t = (prod A) x0 + sum_j (prod A) K_j z_j

Time is split into NCH=8 chunks of CH=8 steps. Host precomputes
  - L[k]  : within-chunk influence of the chunk's measurements (block lower
            triangular, (CH*O, CH*S) per chunk)
  - G[k]  : within-chunk propagation of the chunk entry state ((S, CH*S))
  - W     : influence of measurements on the chunk ENTRY states
  - Gcum / prop matrices : x0 / previous-entry propagation
so the device computes the chunk entry states with a short two-hop matmul
chain (group A = entries 1-3 from (z, x0); group B = entries 4-6 chained off
entry 3), then each chunk's outputs with two matmuls per 128-batch tile.
Chunk 0 reads x0 directly; chunk 7's dependence on entry 7 is folded into
host-composed operators off entry 6 (G[7,i]G[6,7] and G[7,i]L[6,7,j]), so no
third chain hop exists. z/L/W and the output are bf16 (validated ~2e-3 rel
err), the x-propagation path stays f32r. All operand layouts are prepared
host-side: the device performs no transposes and loads no identity matrix.

Entry states live in SBUF tiles of at most 3 chunks (96 partitions) because
matmul operand APs may only start at partitions {0, 32, 64}; each group's
chain-head entry (3, resp. 6) sits at partition 0 so it can feed the next
group's propagation matmul as the moving operand.
"""

import numpy as np
import ml_dtypes

import concourse.bass as bass
import concourse.mybir as mybir
import concourse.tile as tile
from concourse.bass_utils import run_bass_kernel_spmd

S_DIM = 32
O_DIM = 16
T = 64
CH = 8            # timesteps per chunk
NCH = T // CH     # chunks
B = 2048
NCORES = 8
BS = B // NCORES  # batch per core (256)

F32 = mybir.dt.float32
F32R = mybir.dt.float32r
BF16 = mybir.dt.bfloat16

NPBF16 = ml_dtypes.bfloat16

# xs groups: entry states per tile, in row order (chain head first)
XS_GROUPS = [(3, 1, 2), (6, 4, 5)]
XS_BLK = [(0, 1, 2), (3, 4, 5)]
WT_COLS = 2 * 3 * 96  # 576


def _host_mats(F, H, Q, R, P0):
    """Batch-uniform Kalman operators, float64."""
    I = np.eye(S_DIM)
    P = P0
    A_list, K_list = [], []
    for _ in range(T):
        P_pred = F @ P @ F.T + Q
        Sm = H @ P_pred @ H.T + R
        K = P_pred @ H.T @ np.linalg.inv(Sm)
        A = (I - K @ H) @ F
        P = (I - K @ H) @ P_pred
        A_list.append(A)
        K_list.append(K)

    G = np.zeros((NCH, CH, S_DIM, S_DIM))       # [k, i] : x_{8k+i} <- xs_k
    L = np.zeros((NCH, CH, CH, S_DIM, O_DIM))   # [k, i, j] : x_{8k+i} <- z_{8k+j}
    for k in range(NCH):
        for i in range(CH):
            t = CH * k + i
            G[k, i] = A_list[t] @ (G[k, i - 1] if i > 0 else I)
            for j in range(i):
                L[k, i, j] = A_list[t] @ L[k, i - 1, j]
            L[k, i, i] = K_list[t]

    Gcum = np.zeros((NCH, S_DIM, S_DIM))        # xs_k <- x0
    Wfull = np.zeros((NCH, T, S_DIM, O_DIM))    # xs_k <- z_t  (t < 8k)
    Gcum[0] = I
    for k in range(1, NCH):
        Gcum[k] = G[k - 1, CH - 1] @ Gcum[k - 1]
        for t in range(T):
            Wfull[k, t] = G[k - 1, CH - 1] @ Wfull[k - 1, t]
        for j in range(CH):
            Wfull[k, CH * (k - 1) + j] = L[k - 1, CH - 1, j]
    return G, L, Gcum, Wfull


def _gt_rows(Gk):
    """(CH, S, S) [i, s', u] -> [u, i*32+s'] operator layout."""
    return Gk.transpose(2, 0, 1).reshape(32, CH * S_DIM)


def _host_params(F, H, Q, R, P0):
    """Device-layout parameter tensors (shared by all cores)."""
    G, L, Gcum, Wfull = _host_mats(F, H, Q, R, P0)

    # lt[j*16+o, k, i*32+u] = L[k, i, j, u, o]; plane 8 carries chunk 7's
    # folded block-6 weights (G[7,i] @ L[6,7,j])
    lt = np.zeros((CH * O_DIM, NCH + 1, CH * S_DIM))
    lt[:, 0:NCH, :] = L.transpose(2, 4, 0, 1, 3).reshape(
        CH * O_DIM, NCH, CH * S_DIM)
    wg = np.zeros((CH, CH, S_DIM, O_DIM))       # [i, j, s', o]
    for i in range(CH):
        for j in range(CH):
            wg[i, j] = G[7, i] @ Wfull[7, CH * 6 + j]
    lt[:, NCH, :] = wg.transpose(1, 3, 0, 2).reshape(128, CH * S_DIM)

    # wt: flat [128, WT_COLS]; per group g, per block bi, a [128, 96] slab:
    # wt[ti*16+o, off + ji*32 + u] = Wfull[k, bi*8+ti, u, o]
    wt = np.zeros((128, WT_COLS))
    off = 0
    for ks, blks in zip(XS_GROUPS, XS_BLK):
        for bi in blks:
            for ji, k in enumerate(ks):
                blk = Wfull[k, bi * CH:(bi + 1) * CH]          # (CH, S, O)
                wt[:, off + ji * 32:off + (ji + 1) * 32] = (
                    blk.transpose(0, 2, 1).reshape(128, 32))
            off += 96
    assert off == WT_COLS

    # gtab[ji*32+u, p, i*32+s'] = G[XS_GROUPS[p][ji], i, s', u]
    gtab = np.zeros((96, 2, CH * S_DIM))
    for p, ks in enumerate(XS_GROUPS):
        for ji, k in enumerate(ks):
            gtab[ji * 32:(ji + 1) * 32, p] = _gt_rows(G[k])
    # gtc plane 0: G[0] (chunk 0, applied to x0)
    # gtc plane 1: G[7,i] @ G[6,7] (chunk 7, applied to entry 6)
    gtc = np.zeros((32, 2, CH * S_DIM))
    gtc[:, 0] = _gt_rows(G[0])
    gtc[:, 1] = _gt_rows(G[7] @ G[6, CH - 1])

    # gc: x0 -> group-A entry weights [32, 96]
    gc = np.zeros((S_DIM, 96))
    for ji, k in enumerate(XS_GROUPS[0]):
        gc[:, ji * 32:(ji + 1) * 32] = Gcum[k].T

    # pb: entry 3 -> group-B entries [32, 96]
    fullG = [G[k, CH - 1] for k in range(NCH)]
    prop = {4: fullG[3], 5: fullG[4] @ fullG[3],
            6: fullG[5] @ fullG[4] @ fullG[3]}
    pb = np.zeros((S_DIM, 96))
    for ji, k in enumerate(XS_GROUPS[1]):
        pb[:, ji * 32:(ji + 1) * 32] = prop[k].T

    return (lt.astype(NPBF16), wt.astype(NPBF16),
            gtab.astype(np.float32), gtc.astype(np.float32),
            np.concatenate([gc, pb], axis=1).astype(np.float32))


def _host_data(state0, measurements, core):
    """Per-core data tensors in device layout."""
    zc = measurements[core * BS:(core + 1) * BS]   # (256, T, O) f32
    zc = zc.reshape(2, 128, NCH, CH, O_DIM)        # (h, b, k, i, o)
    zt = zc.transpose(3, 4, 2, 0, 1).reshape(CH * O_DIM, NCH, 2, 128)
    x0t = state0[core * BS:(core + 1) * BS].T      # (32, 256): [s, h*128+b]
    return zt.astype(NPBF16), np.ascontiguousarray(x0t.astype(np.float32))


def build_nc(split_waits=True):
    nc = bass.Bass("TRN2", target_bir_lowering=False, debug=False,
                   num_devices=NCORES)

    zt_d = nc.dram_tensor("zt", (128, NCH, 2, 128), BF16, kind="ExternalInput")
    lt_d = nc.dram_tensor("lt", (128, NCH + 1, CH * S_DIM), BF16,
                          kind="ExternalInput")
    wt_d = nc.dram_tensor("wt", (128, WT_COLS), BF16, kind="ExternalInput")
    gtab_d = nc.dram_tensor("gtab", (96, 2, CH * S_DIM), F32R,
                            kind="ExternalInput")
    gtc_d = nc.dram_tensor("gtc", (32, 2, CH * S_DIM), F32R,
                           kind="ExternalInput")
    # pf: x0t (cols 0:256) ++ gc (256:352) ++ pb (352:448)
    pf_d = nc.dram_tensor("pf", (S_DIM, 448), F32R, kind="ExternalInput")
    out_d = nc.dram_tensor("out", (128, 2, NCH, CH * S_DIM), BF16,
                           kind="ExternalOutput")

    with tile.TileContext(nc) as tc:
        with (
            tc.tile_pool(name="const", bufs=1) as const,
            tc.tile_pool(name="xs_sb", bufs=1) as xs_p,
            tc.tile_pool(name="souts", bufs=1) as s_p,
            tc.tile_pool(name="psxs", bufs=1, space="PSUM") as ps_x,
            tc.tile_pool(name="psc", bufs=1, space="PSUM") as ps_c,
            tc.tile_pool(name="pss", bufs=1, space="PSUM") as ps_s,
        ):
            # --- input DMAs, spread across the three DGE queues; halves of
            # zt/lt land separately so chunk 0-3 compute starts early ---
            zt = const.tile([128, NCH, 2, 128], BF16)
            lt = const.tile([128, NCH + 1, CH * S_DIM], BF16)
            wt = const.tile([128, WT_COLS], BF16)
            gtab = const.tile([96, 2, CH * S_DIM], F32R)
            gtc = const.tile([32, 2, CH * S_DIM], F32R)
            pf = const.tile([S_DIM, 448], F32R)

            nc.sync.dma_start(zt[:, 0:3, :, :], zt_d[:, 0:3, :, :])
            nc.scalar.dma_start(lt[:, 0:2, :], lt_d[:, 0:2, :])
            nc.gpsimd.dma_start(wt[:], wt_d[:])
            nc.sync.dma_start(zt[:, 3:6, :, :], zt_d[:, 3:6, :, :])
            nc.scalar.dma_start(pf[:], pf_d[:])
            nc.gpsimd.dma_start(gtab[:], gtab_d[:])
            nc.sync.dma_start(zt[:, 6:8, :, :], zt_d[:, 6:8, :, :])
            nc.scalar.dma_start(gtc[:], gtc_d[:])
            nc.gpsimd.dma_start(lt[:, 2:9, :], lt_d[:, 2:9, :])

            # warm the Activation engine's function table off the critical
            # path (first act op otherwise pays ~1.3us mid-kernel); emitted
            # after Act's input DMAs so it does not stall their dispatch
            warm = const.tile([1, 2], F32)
            nc.vector.memset(warm[:, 0:1], 0.0)
            nc.scalar.copy(warm[:, 1:2], warm[:, 0:1])

            # --- tiles ---
            # entry-state groups A and B share one PSUM bank (two sequential
            # accumulation groups on disjoint column regions)
            xs_ps = ps_x.tile([96, 512], F32, name="xspsAB")
            xs_regions = [xs_ps[:, 0:256], xs_ps[:, 256:512]]
            xs_sb = [xs_p.tile([96, 256], F32R, name="xs0"),
                     xs_p.tile([96, 256], F32R, name="xs1")]
            # PSUM: chunks 0+1 share one bank per half (retired as a pair);
            # chunk 2 reuses those banks after the pair's copies drain (pool
            # WAR dep); chunks 3-7 each own a bank so every late chunk
            # retires singly and the copies/stores spread across the window.
            pair_ps = {h: ps_c.tile([128, 2, CH * S_DIM], F32, name=f"cps{h}")
                       for h in range(2)}
            pair_sb = s_p.tile([128, 2, 2, CH * S_DIM], BF16, name="souts01")
            c2_ps = {h: ps_c.tile([128, CH * S_DIM], F32, name=f"cps{h}")
                     for h in range(2)}
            sing_ps = {k: ps_s.tile([128, 2, CH * S_DIM], F32, name=f"cs{k}")
                       for k in (3, 4, 5, 6, 7)}
            sing_sb = {k: s_p.tile([128, 2, CH * S_DIM], BF16,
                                   name=f"souts_s{k}") for k in (2, 3, 4, 5, 6, 7)}

            # --- matmul emitters ---
            def xsrc(k):
                if k == 7:
                    return xs_sb[1], 0, gtc, 1
                g = 0 if k <= 3 else 1
                return xs_sb[g], XS_GROUPS[g].index(k) * 32, gtab, g

            def xmm_ops(k, h):
                if k == 0:
                    return pf[:, h * 128:(h + 1) * 128], gtc[:, 0, :]
                xsb, roff, gtt, plane = xsrc(k)
                return (xsb[roff:roff + 32, h * 128:(h + 1) * 128],
                        gtt[roff:roff + 32, plane, :])

            def c_region(k, h):
                if k >= 3:
                    return sing_ps[k][:, h, :]
                if k == 2:
                    return c2_ps[h][:]
                return pair_ps[h][:, k, :]

            # PSUM start/stop is per 2KB zero-region (= one bank tile), so
            # each tile forms ONE accumulation group: start on the first
            # matmul that touches it, stop on the last.
            def zmm(k, h, first=None):
                if first is None:
                    first = (h == 0) if k >= 3 else (k % 2 == 0 or k == 2)
                nc.tensor.matmul(c_region(k, h), zt[:, k, h, :], lt[:, k, :],
                                 start=first, stop=False)

            def zmm7fold(h):
                # chunk 7's folded block-6 term: z chunk 6 against
                # G[7,i] @ L[6,7,j] (lt plane 8)
                nc.tensor.matmul(c_region(7, h), zt[:, 6, h, :],
                                 lt[:, NCH, :], start=False, stop=False)

            def xmm(k, h):
                last = (h == 1) if k >= 3 else (k % 2 == 1 or k == 2)
                lhsT, rhs = xmm_ops(k, h)
                nc.tensor.matmul(c_region(k, h), lhsT, rhs,
                                 start=False, stop=last)

            def xs_zmm(g, j, first):
                nc.tensor.matmul(
                    xs_regions[g],
                    wt[:, (3 * g + j) * 96:(3 * g + j + 1) * 96],
                    zt[:, XS_BLK[g][j], :, :].rearrange("p h b -> p (h b)"),
                    start=first, stop=False,
                )

            def xs_head(g):
                nc.tensor.matmul(
                    xs_regions[g],
                    pf[:, 256 + g * 96:256 + (g + 1) * 96],
                    pf[:, 0:256] if g == 0 else xs_sb[0][0:32, :],
                    start=False, stop=True,
                )

            # --- instruction stream: xs chain first (it gates the most
            # downstream work), zmms fill the gaps, then chunks retire one
            # at a time so the copies/stores stream steadily on DVE/Act,
            # ending with chunk 7 (smallest store) for the shortest tail ---
            for j in range(3):
                xs_zmm(0, j, j == 0)          # needs zt[0:3] + wt
            xs_head(0)                        # needs pf
            nc.vector.tensor_copy(xs_sb[0][:], xs_regions[0])
            zmm(0, 0), zmm(0, 1)              # needs zt[0:3] + lt[0:2]
            zmm(1, 0), zmm(1, 1)
            for j in range(3):
                xs_zmm(1, j, j == 0)          # needs zt[3:6]
            zmm(3, 0), zmm(3, 1)              # fill while xs_A copy lands
            zmm(4, 0), zmm(4, 1)
            xs_head(1)                        # needs xs_A in SBUF
            nc.vector.tensor_copy(xs_sb[1][:], xs_regions[1])
            xmm(0, 0), xmm(0, 1)              # needs gtc
            xmm(1, 0), xmm(1, 1)              # needs xs_A; pair 0+1 retires
            nc.vector.tensor_copy(pair_sb[:, 0, :, :], pair_ps[0][:])
            nc.scalar.copy(pair_sb[:, 1, :, :], pair_ps[1][:])
            nc.sync.dma_start(out_d[:, :, 0:2, :], pair_sb[:])
            xmm(3, 0), xmm(3, 1)              # chunk 3 retires
            nc.vector.tensor_copy(sing_sb[3][:, 0, :], sing_ps[3][:, 0, :])
            nc.scalar.copy(sing_sb[3][:, 1, :], sing_ps[3][:, 1, :])
            nc.gpsimd.dma_start(out_d[:, :, 3, :], sing_sb[3][:])
            zmm(5, 0), zmm(5, 1)
            xmm(4, 0), xmm(4, 1)              # needs xs_B; chunk 4 retires
            nc.vector.tensor_copy(sing_sb[4][:, 0, :], sing_ps[4][:, 0, :])
            nc.scalar.copy(sing_sb[4][:, 1, :], sing_ps[4][:, 1, :])
            nc.sync.dma_start(out_d[:, :, 4, :], sing_sb[4][:])
            zmm(6, 0), zmm(6, 1)
            xmm(5, 0), xmm(5, 1)              # chunk 5 retires
            nc.vector.tensor_copy(sing_sb[5][:, 0, :], sing_ps[5][:, 0, :])
            nc.scalar.copy(sing_sb[5][:, 1, :], sing_ps[5][:, 1, :])
            nc.gpsimd.dma_start(out_d[:, :, 5, :], sing_sb[5][:])
            zmm(7, 0), zmm(7, 1)
            zmm7fold(0), zmm7fold(1)
            xmm(6, 0), xmm(6, 1)              # chunk 6 retires
            nc.vector.tensor_copy(sing_sb[6][:, 0, :], sing_ps[6][:, 0, :])
            nc.scalar.copy(sing_sb[6][:, 1, :], sing_ps[6][:, 1, :])
            nc.sync.dma_start(out_d[:, :, 6, :], sing_sb[6][:])
            zmm(2, 0), zmm(2, 1)              # needs pair banks freed
            xmm(2, 0), xmm(2, 1)              # chunk 2 retires
            nc.vector.tensor_copy(sing_sb[2][:, 0, :], c2_ps[0][:])
            nc.scalar.copy(sing_sb[2][:, 1, :], c2_ps[1][:])
            nc.gpsimd.dma_start(out_d[:, :, 2, :], sing_sb[2][:])
            xmm(7, 0), xmm(7, 1)              # chunk 7 (folded entry 7)
            nc.vector.tensor_copy(sing_sb[7][:, 0, :], sing_ps[7][:, 0, :])
            nc.scalar.copy(sing_sb[7][:, 1, :], sing_ps[7][:, 1, :])
            # final chunk split by half across two queues for the short tail
            nc.sync.dma_start(out_d[:, 0, 7, :], sing_sb[7][:, 0, :])
            nc.scalar.dma_start(out_d[:, 1, 7, :], sing_sb[7][:, 1, :])

    if split_waits:
        _split_matmul_waits(nc)
    return nc


def _split_matmul_waits(nc, max_waits=1):
    """Walrus lowers f32/f32r matmuls through the LDWEIGHTS template, which
    supports fewer sync-wait slots than Tile may emit. Move excess waits onto
    a PE NoOp inserted right before the offending matmul."""
    for f in nc.m.functions:
        for blk in f.blocks:
            insts = list(blk.instructions)
            out = []
            for inst in insts:
                si = inst.sync_info
                if si is not None and si.on_wait and len(si.on_wait) > max_waits:
                    waits = list(si.on_wait)
                    carry, keep = waits[:-max_waits], waits[-max_waits:]
                    for w in carry:
                        nop = mybir.InstNoOp(
                            name=nc.get_next_instruction_name(),
                            sync_info=mybir.SyncInfo(on_wait=[w], on_update=[]),
                            bass_nofuse=True,
                            engine=inst.engine,
                        )
                        out.append(nop)
                    inst.sync_info = mybir.SyncInfo(
                        on_wait=keep, on_update=list(si.on_update or [])
                    )
                out.append(inst)
            if len(out) != len(insts):
                blk.instructions = out


_CACHE = {}


def kernel(state0, cov0, measurements, F, H, Q, R, _trace=False):
    state0 = np.ascontiguousarray(np.asarray(state0, np.float32))
    measurements = np.ascontiguousarray(np.asarray(measurements, np.float32))
    lt, wt, gtab, gtc, gc = _host_params(
        np.asarray(F, np.float64), np.asarray(H, np.float64),
        np.asarray(Q, np.float64), np.asarray(R, np.float64),
        np.asarray(cov0, np.float64)[0],
    )

    if "nc" not in _CACHE:
        _CACHE["nc"] = build_nc()
    nc = _CACHE["nc"]

    in_maps = []
    for c in range(NCORES):
        zt, x0t = _host_data(state0, measurements, c)
        pf = np.ascontiguousarray(np.concatenate([x0t, gc], axis=1))
        in_maps.append({"zt": zt, "lt": lt, "wt": wt, "gtab": gtab,
                        "gtc": gtc, "pf": pf})

    res = run_bass_kernel_spmd(nc, in_maps, core_ids=list(range(NCORES)),
                               trace=_trace)
    outs = []
    for c in range(NCORES):
        o = np.asarray(res.results[c]["out"])  # (128, 2, NCH, CH*S) bf16
        o = o.astype(np.float32).reshape(128, 2, NCH, CH, S_DIM)
        o = o.transpose(1, 0, 2, 3, 4).reshape(BS, T, S_DIM)
        outs.append(o)
    out = np.concatenate(outs, axis=0)
    if _trace:
        kernel._last_result = res
    return out
